# revision 28
# baseline (speedup 1.0000x reference)
"""Trainium2 Bass kernel for nn_CrossAttention (16x512x64x64, 8 heads x 64).

Math (exact algebraic restructuring of the reference; see baseline notes):
  The tiled-k/v convs are rank-1, so everything except q = to_q_w @ x
  collapses to per-sample vectors computed on HOST from weights+y:
    w[b,hd]  = sum_j softmax_j(rs_k[hd]*ky[b,j]) * vy[b,j]
    W2[o,h]  = scale * sum_e out_w[o,64h+e] * rs_v[64h+e]
  Device computes, per sample:
    q[he,n]   = to_q_w @ x              (PE, bf16, [he,n] layout)
    e         = exp(q)                  (ACT)
    num/den   = mask-matmul over he     (PE: lhsT = [w-masked | 1-masked])
    s[h,n]    = num/den                 (DVE)
    mm[o,n]   = W2 @ s                  (PE, K=8)
    stats     = sum/sumsq of mm rows    (accum_out on the PSUM->SBUF copy + DVE)
    out       = A(o)*mm + B(o)          (GroupNorm affine, GPSIMD, in-place)

Sharding: data-parallel over batch, 2 samples/core, 8 cores, no collectives.
All I/O in bf16 (host converts); weights pre-transposed on host.
"""

import numpy as np
import ml_dtypes

import concourse.bass as bass
import concourse.mybir as mybir
import concourse.tile as tile
from concourse import bacc
from concourse.bass import ts
from concourse.bass_utils import run_bass_kernel_spmd

B, C, N = 16, 512, 4096
DIMY = 768
HEADS, DHEAD = 8, 64
NCORES = 8
BPC = B // NCORES
SCALE = DHEAD ** -0.5
EPS = 1e-5
F32 = mybir.dt.float32
BF16 = mybir.dt.bfloat16
AX = mybir.AxisListType.X
AF = mybir.ActivationFunctionType
OP = mybir.AluOpType
BF16NP = ml_dtypes.bfloat16


def build_nc(use_f32r=True):
    nc = bacc.Bacc()
    xd = nc.dram_tensor("xb", [BPC, 4, 128, N], BF16, kind="ExternalInput")
    qwtd = nc.dram_tensor("qwt", [128, 4, C], BF16, kind="ExternalInput")
    wmd = nc.dram_tensor("wm", [128, BPC, 4, 40], BF16, kind="ExternalInput")
    w2td = nc.dram_tensor("w2t", [HEADS, 4, 128], BF16, kind="ExternalInput")
    gcd = nc.dram_tensor("gcols", [128, 4, 3], F32, kind="ExternalInput")
    outd = nc.dram_tensor("out", [BPC, 4, 128, N], BF16, kind="ExternalOutput")

    from contextlib import ExitStack

    with tile.TileContext(nc) as tc, ExitStack() as ctx:
        persist = ctx.enter_context(tc.tile_pool(name="persist", bufs=1))
        xp = ctx.enter_context(tc.tile_pool(name="xp", bufs=8))
        ep = ctx.enter_context(tc.tile_pool(name="ep", bufs=10))
        sttp = ctx.enter_context(tc.tile_pool(name="sttp", bufs=4))
        rdp = ctx.enter_context(tc.tile_pool(name="rdp", bufs=4))
        mmp = ctx.enter_context(tc.tile_pool(name="mmp", bufs=2))
        accp = ctx.enter_context(tc.tile_pool(name="accp", bufs=4))
        stgp = ctx.enter_context(tc.tile_pool(name="stgp", bufs=3))
        smallp = ctx.enter_context(tc.tile_pool(name="smallp", bufs=10))
        rowp = ctx.enter_context(tc.tile_pool(name="rowp", bufs=8))
        psqp = ctx.enter_context(tc.tile_pool(name="psqp", bufs=3, space="PSUM"))
        ps16p = ctx.enter_context(tc.tile_pool(name="ps16p", bufs=2, space="PSUM"))
        psfp = ctx.enter_context(tc.tile_pool(name="psfp", bufs=3, space="PSUM"))

        # ---------------- weights / constants ----------------
        qwt = persist.tile([128, 4, C], BF16, tag="qwt")
        nc.sync.dma_start(out=qwt, in_=qwtd[:, :, :])
        wm = persist.tile([128, BPC, 4, 40], BF16, tag="wm")
        w2t = persist.tile([HEADS, 4, 128], BF16, tag="w2t")
        gcols = persist.tile([128, 4, 3], F32, tag="gcols")
        outb_c = gcols[:, :, 0]
        gng_c = gcols[:, :, 1]
        gnb_c = gcols[:, :, 2]

        ones_col = persist.tile([128, 1], F32, tag="ones")
        nc.vector.memset(ones_col, 1.0)
        ones_row = persist.tile([1, 128], F32, tag="onesr")
        nc.vector.memset(ones_row, 1.0)
        zero_col = persist.tile([128, 1], F32, tag="zero")
        nc.vector.memset(zero_col, 0.0)
        nc.const_aps.aps[(F32, 0.0)] = zero_col[:, :]
        eps_col = persist.tile([128, 1], F32, tag="eps")
        nc.vector.memset(eps_col, EPS)
        nc.const_aps.aps[(F32, EPS)] = eps_col[:, :]

        # x tiles: sample 0 in halves (faster lead-in), rest whole
        xs = {}
        for s in range(BPC):
            for ct in range(4):
                xc = xp.tile([128, N], BF16, tag="xc")
                xs[(s, ct)] = xc
        for ct in range(4):
            nc.sync.dma_start(out=xs[(0, ct)][:, 0:1024], in_=xd[0, ct, :, 0:1024])
        # non-critical weights after the first x quarter (needed ~10us in)
        nc.sync.dma_start(out=wm, in_=wmd[:, :, :, :])
        nc.sync.dma_start(out=w2t, in_=w2td[:, :, :])
        nc.sync.dma_start(out=gcols, in_=gcd[:, :, :])
        for q in range(1, 4):
            for ct in range(4):
                nc.sync.dma_start(
                    out=xs[(0, ct)][:, ts(q, 1024)], in_=xd[0, ct, :, ts(q, 1024)]
                )
        for s in range(1, BPC):
            for ct in range(4):
                nc.sync.dma_start(out=xs[(s, ct)], in_=xd[s, ct, :, :])

        mmbs, statss = {}, {}

        def emit_q(s, g):
            es = []
            for ot in range(4):
                psq = psqp.tile([128, 512], F32, tag="psq")
                for ct in range(4):
                    nc.tensor.matmul(
                        psq,
                        lhsT=qwt[:, ct, ts(ot, 128)],
                        rhs=xs[(s, ct)][:, ts(g, 512)],
                        start=(ct == 0),
                        stop=(ct == 3),
                    )
                e = ep.tile([128, 512], BF16, tag="e")
                nc.scalar.activation(out=e, in_=psq, func=AF.Exp)
                es.append(e)
            return es

        def emit_redfin(s, g, es):
            ps16 = ps16p.tile([40, 512], F32, tag="ps16")
            for ot in range(4):
                nc.tensor.matmul(
                    ps16,
                    lhsT=wm[:, s, ot, :],
                    rhs=es[ot],
                    start=(ot == 0),
                    stop=(ot == 3),
                )
            den_sb = rdp.tile([8, 512], F32, tag="densb")
            nc.scalar.copy(out=den_sb, in_=ps16[32:40, :])
            rden = rdp.tile([8, 512], F32, tag="rden")
            nc.vector.reciprocal_approx_fast(out=rden, in_=den_sb)
            stt = sttp.tile([8, 512], BF16, tag="stt")
            nc.vector.tensor_mul(stt, ps16[0:8, :], rden)
            mmb, stats = mmbs[s], statss[s]
            for ot in range(4):
                psf = psfp.tile([128, 512], F32, tag="psf")
                nc.tensor.matmul(
                    psf, lhsT=w2t[:, ot, :], rhs=stt, start=True, stop=True
                )
                dst = mmb[:, ot, ts(g, 512)]
                if (ot + g) % 2 == 0:
                    nc.scalar.copy(out=dst, in_=psf)
                else:
                    nc.vector.tensor_copy(dst, psf)
                nc.vector.bn_stats(out=stats[:, ot, g], in_=psf)

        def emit_epilogue(s):
            mmb, stats = mmbs[s], statss[s]
            mv8 = smallp.tile([128, 8], F32, tag="mv8")
            for ot in range(4):
                mv = smallp.tile([128, 2], F32, tag="mv")
                nc.vector.bn_aggr(out=mv, in_=stats[:, ot])
                m_ = mv8[:, ot : ot + 1]
                nc.vector.tensor_add(m_, mv[:, 0:1], outb_c[:, ot : ot + 1])
                msq = smallp.tile([128, 1], F32, tag="msq")
                nc.vector.tensor_mul(msq, m_, m_)
                nc.vector.tensor_add(mv8[:, 4 + ot : 5 + ot], mv[:, 1:2], msq)

            ps_tot = ps16p.tile([1, 8], F32, tag="ps16")
            nc.tensor.matmul(ps_tot, lhsT=ones_col, rhs=mv8, start=True, stop=True)
            rowt8 = rowp.tile([1, 8], F32, tag="rowt8")
            nc.scalar.copy(out=rowt8, in_=ps_tot)
            tt = rowp.tile([1, 2], F32, tag="tt")
            nc.vector.reduce_sum(
                out=tt, in_=rowt8.rearrange("p (a b) -> p a b", a=2), axis=AX
            )
            tt2 = rowp.tile([1, 2], F32, tag="tt2")
            nc.scalar.mul(out=tt2, in_=tt, mul=1.0 / C)  # {mu, E2}
            msq = rowp.tile([1, 1], F32, tag="msq")
            nc.vector.tensor_mul(msq, tt2[:, 0:1], tt2[:, 0:1])
            var = rowp.tile([1, 1], F32, tag="var")
            nc.vector.tensor_sub(var, tt2[:, 1:2], msq)
            sd = rowp.tile([1, 1], F32, tag="sd")
            nc.scalar.activation(out=sd, in_=var, func=AF.Sqrt, bias=EPS)
            rstd = rowp.tile([1, 1], F32, tag="rstd")
            nc.vector.reciprocal(rstd, sd)
            murow = rowp.tile([1, 2], F32, tag="murow")
            nc.vector.tensor_copy(murow[:, 0:1], tt2[:, 0:1])
            nc.vector.tensor_copy(murow[:, 1:2], rstd)
            ps_b = ps16p.tile([128, 2], F32, tag="ps16")
            nc.tensor.matmul(ps_b, lhsT=ones_row, rhs=murow, start=True, stop=True)
            msb = smallp.tile([128, 2], F32, tag="msb")
            nc.scalar.copy(out=msb, in_=ps_b)

            a_col = smallp.tile([128, 4], F32, tag="acol")
            nc.vector.tensor_scalar_mul(a_col, gng_c, msb[:, 1:2])
            t1 = smallp.tile([128, 4], F32, tag="t1")
            nc.vector.tensor_scalar(
                out=t1, in0=outb_c, scalar1=msb[:, 0:1], scalar2=None,
                op0=OP.subtract,
            )
            t2 = smallp.tile([128, 4], F32, tag="t2")
            nc.vector.tensor_mul(t2, a_col, t1)
            b2 = smallp.tile([128, 4], F32, tag="b2")
            nc.vector.tensor_add(b2, t2, gnb_c)

            if s == BPC - 1:
                # tail-critical: affine in halves, DMA split across two rings
                for ot in range(4):
                    for h in range(2):
                        stg = stgp.tile([128, 2048], BF16, tag="stgh")
                        nc.vector.tensor_scalar(
                            out=stg, in0=mmb[:, ot, ts(h, 2048)],
                            scalar1=a_col[:, ot : ot + 1],
                            scalar2=b2[:, ot : ot + 1],
                            op0=OP.mult, op1=OP.add,
                        )
                        eng = nc.sync if (2 * ot + h) % 2 == 0 else nc.gpsimd
                        eng.dma_start(
                            out=outd[s, ot, :, ts(h, 2048)], in_=stg
                        )
            else:
                for ot in range(4):
                    stg = stgp.tile([128, N], BF16, tag="stg")
                    nc.vector.tensor_scalar(
                        out=stg, in0=mmb[:, ot, :],
                        scalar1=a_col[:, ot : ot + 1], scalar2=b2[:, ot : ot + 1],
                        op0=OP.mult, op1=OP.add,
                    )
                    nc.sync.dma_start(out=outd[s, ot, :, :], in_=stg)

        # software pipeline: PE runs q(g+1) while ACT/DVE produce e/stt for g
        pend = None
        for s in range(BPC):
            for g in range(8):
                if g == 0:
                    mmb_t = mmp.tile([128, 4, N], BF16, tag="mmb")
                    stats_t = accp.tile([128, 4, 8, 6], F32, tag="stats")
                    mmbs[s] = mmb_t
                    statss[s] = stats_t
                es = emit_q(s, g)
                if pend is not None:
                    ps_, pg_, pes_ = pend
                    emit_redfin(ps_, pg_, pes_)
                    if pg_ == 7:
                        emit_epilogue(ps_)
                pend = (s, g, es)
        ps_, pg_, pes_ = pend
        emit_redfin(ps_, pg_, pes_)
        emit_epilogue(ps_)

    nc.finalize()
    return nc


_NC_CACHE = {}


def _get_nc(use_f32r=True):
    if "nc" not in _NC_CACHE:
        _NC_CACHE["nc"] = build_nc()
    return _NC_CACHE["nc"]


def make_in_maps(inputs):
    f32 = np.float32
    x = np.ascontiguousarray(inputs["x"], dtype=f32).reshape(B, C, N)
    y = np.ascontiguousarray(inputs["y"], dtype=f32).reshape(B, DIMY)
    k_w = np.asarray(inputs["k_w"], f32)
    v_w = np.asarray(inputs["v_w"], f32)
    to_q_w = np.asarray(inputs["to_q_w"], f32)
    to_k_w = np.asarray(inputs["to_k_w"], f32)
    to_v_w = np.asarray(inputs["to_v_w"], f32)
    out_w = np.asarray(inputs["out_w"], f32)
    out_b = np.asarray(inputs["out_b"], f32)
    gn_g = np.asarray(inputs["gn_g"], f32)
    gn_b = np.asarray(inputs["gn_b"], f32)

    # host precompute: per-sample softmax-weighted value vector w[b,hd], and
    # the collapsed output weight W2[o,h] (all O(weights)/O(y) work)
    ky = y @ k_w.T                                   # [B, C]
    vy = y @ v_w.T
    rs_k = to_k_w.sum(1)                             # [C]
    rs_v = to_v_w.sum(1)
    ez = np.exp(rs_k[None, :, None] * ky[:, None, :])          # [B, hd, j]
    wvec = (ez * vy[:, None, :]).sum(-1) / ez.sum(-1)          # [B, C]
    W2 = SCALE * (
        out_w.reshape(C, HEADS, DHEAD) * rs_v.reshape(HEADS, DHEAD)[None]
    ).sum(-1)                                        # [C, 8]

    # reduction masks: [B, 4ot, 128p, 40]; col j: w if head==j, col 32+j: 1
    # (cols 8-31 zero-padded so num lands at psum partitions 0-7 and den at
    # 32-39 -- engine partition reads must be 32-aligned)
    hd = np.arange(C)
    head = hd // DHEAD
    ot_i, p_i = hd // 128, hd % 128
    wmask = np.zeros((B, 4, 128, 40), f32)
    wmask[:, ot_i, p_i, head] = wvec
    wmask[:, ot_i, p_i, 32 + head] = 1.0

    qwt = np.ascontiguousarray(
        to_q_w.T.reshape(4, 128, C).transpose(1, 0, 2)
    ).astype(BF16NP)                                 # [128p, 4ct, 512o]
    w2t = np.ascontiguousarray(W2.T.reshape(HEADS, 4, 128)).astype(BF16NP)
    gcols = np.ascontiguousarray(
        np.stack(
            [out_b.reshape(4, 128).T, gn_g.reshape(4, 128).T,
             gn_b.reshape(4, 128).T],
            axis=2,
        )
    ).astype(f32)                                    # [128, 4, 3]

    xb = x.reshape(B, 4, 128, N).astype(BF16NP)
    in_maps = []
    for core in range(NCORES):
        s0 = core * BPC
        m = {
            "xb": np.ascontiguousarray(xb[s0 : s0 + BPC]),
            "wm": np.ascontiguousarray(
                wmask[s0 : s0 + BPC].transpose(2, 0, 1, 3)
            ).astype(BF16NP),                        # [128, BPC, 4, 16]
            "qwt": qwt,
            "w2t": w2t,
            "gcols": gcols,
        }
        in_maps.append(m)
    return in_maps


def kernel(**inputs):
    nc = _get_nc()
    res = run_bass_kernel_spmd(nc, make_in_maps(inputs), list(range(NCORES)))
    out = np.concatenate([r["out"] for r in res.results], axis=0)  # [B,4,128,N]
    return out.reshape(B, C, N).astype(np.float32).reshape(B, C, 64, 64)


if __name__ == "__main__":
    rng = np.random.default_rng(0)
    inputs = {
        "x": rng.standard_normal((B, C, 64, 64), dtype=np.float32),
        "y": rng.standard_normal((B, 1, 1, DIMY), dtype=np.float32),
        "k_w": rng.standard_normal((C, DIMY), dtype=np.float32) * 0.02,
        "v_w": rng.standard_normal((C, DIMY), dtype=np.float32) * 0.02,
        "to_q_w": rng.standard_normal((C, C), dtype=np.float32) * 0.02,
        "to_k_w": rng.standard_normal((C, C), dtype=np.float32) * 0.02,
        "to_v_w": rng.standard_normal((C, C), dtype=np.float32) * 0.02,
        "out_w": rng.standard_normal((C, C), dtype=np.float32) * 0.02,
        "out_b": np.zeros(C, np.float32),
        "gn_g": np.ones(C, np.float32),
        "gn_b": np.zeros(C, np.float32),
    }
    out = kernel(**inputs)
    print("kernel ran, out shape", out.shape, "std", out.std())


# revision 29
# speedup vs baseline: 1.0031x; 1.0031x over previous
"""Trainium2 Bass kernel for nn_CrossAttention (16x512x64x64, 8 heads x 64).

Math (exact algebraic restructuring of the reference; see baseline notes):
  The tiled-k/v convs are rank-1, so everything except q = to_q_w @ x
  collapses to per-sample vectors computed on HOST from weights+y:
    w[b,hd]  = sum_j softmax_j(rs_k[hd]*ky[b,j]) * vy[b,j]
    W2[o,h]  = scale * sum_e out_w[o,64h+e] * rs_v[64h+e]
  Device computes, per sample:
    q[he,n]   = to_q_w @ x              (PE, bf16, [he,n] layout)
    e         = exp(q)                  (ACT)
    num/den   = mask-matmul over he     (PE: lhsT = [w-masked | 1-masked])
    s[h,n]    = num/den                 (DVE)
    mm[o,n]   = W2 @ s                  (PE, K=8)
    stats     = sum/sumsq of mm rows    (accum_out on the PSUM->SBUF copy + DVE)
    out       = A(o)*mm + B(o)          (GroupNorm affine, GPSIMD, in-place)

Sharding: data-parallel over batch, 2 samples/core, 8 cores, no collectives.
All I/O in bf16 (host converts); weights pre-transposed on host.
"""

import numpy as np
import ml_dtypes

import concourse.bass as bass
import concourse.mybir as mybir
import concourse.tile as tile
from concourse import bacc
from concourse.bass import ts
from concourse.bass_utils import run_bass_kernel_spmd

B, C, N = 16, 512, 4096
DIMY = 768
HEADS, DHEAD = 8, 64
NCORES = 8
BPC = B // NCORES
SCALE = DHEAD ** -0.5
EPS = 1e-5
F32 = mybir.dt.float32
BF16 = mybir.dt.bfloat16
AX = mybir.AxisListType.X
AF = mybir.ActivationFunctionType
OP = mybir.AluOpType
BF16NP = ml_dtypes.bfloat16


def build_nc(use_f32r=True):
    nc = bacc.Bacc()
    xd = nc.dram_tensor("xb", [BPC, 4, 128, N], BF16, kind="ExternalInput")
    qwtd = nc.dram_tensor("qwt", [128, 4, C], BF16, kind="ExternalInput")
    wmd = nc.dram_tensor("wm", [128, BPC, 4, 40], BF16, kind="ExternalInput")
    w2td = nc.dram_tensor("w2t", [HEADS, 4, 128], BF16, kind="ExternalInput")
    gcd = nc.dram_tensor("gcols", [128, 4, 3], F32, kind="ExternalInput")
    outd = nc.dram_tensor("out", [BPC, 4, 128, N], BF16, kind="ExternalOutput")

    from contextlib import ExitStack

    with tile.TileContext(nc) as tc, ExitStack() as ctx:
        persist = ctx.enter_context(tc.tile_pool(name="persist", bufs=1))
        xp = ctx.enter_context(tc.tile_pool(name="xp", bufs=8))
        ep = ctx.enter_context(tc.tile_pool(name="ep", bufs=10))
        sttp = ctx.enter_context(tc.tile_pool(name="sttp", bufs=4))
        rdp = ctx.enter_context(tc.tile_pool(name="rdp", bufs=4))
        mmp = ctx.enter_context(tc.tile_pool(name="mmp", bufs=2))
        accp = ctx.enter_context(tc.tile_pool(name="accp", bufs=4))
        stgp = ctx.enter_context(tc.tile_pool(name="stgp", bufs=4))
        smallp = ctx.enter_context(tc.tile_pool(name="smallp", bufs=10))
        rowp = ctx.enter_context(tc.tile_pool(name="rowp", bufs=8))
        psqp = ctx.enter_context(tc.tile_pool(name="psqp", bufs=3, space="PSUM"))
        ps16p = ctx.enter_context(tc.tile_pool(name="ps16p", bufs=2, space="PSUM"))
        psfp = ctx.enter_context(tc.tile_pool(name="psfp", bufs=3, space="PSUM"))

        # ---------------- weights / constants ----------------
        qwt = persist.tile([128, 4, C], BF16, tag="qwt")
        nc.sync.dma_start(out=qwt, in_=qwtd[:, :, :])
        wm = persist.tile([128, BPC, 4, 40], BF16, tag="wm")
        nc.sync.dma_start(out=wm, in_=wmd[:, :, :, :])
        w2t = persist.tile([HEADS, 4, 128], BF16, tag="w2t")
        nc.sync.dma_start(out=w2t, in_=w2td[:, :, :])
        gcols = persist.tile([128, 4, 3], F32, tag="gcols")
        nc.sync.dma_start(out=gcols, in_=gcd[:, :, :])
        outb_c = gcols[:, :, 0]
        gng_c = gcols[:, :, 1]
        gnb_c = gcols[:, :, 2]

        ones_col = persist.tile([128, 1], F32, tag="ones")
        nc.vector.memset(ones_col, 1.0)
        ones_row = persist.tile([1, 128], F32, tag="onesr")
        nc.vector.memset(ones_row, 1.0)
        zero_col = persist.tile([128, 1], F32, tag="zero")
        nc.vector.memset(zero_col, 0.0)
        nc.const_aps.aps[(F32, 0.0)] = zero_col[:, :]
        eps_col = persist.tile([128, 1], F32, tag="eps")
        nc.vector.memset(eps_col, EPS)
        nc.const_aps.aps[(F32, EPS)] = eps_col[:, :]

        # x tiles: sample 0 in halves (faster lead-in), rest whole
        xs = {}
        for s in range(BPC):
            for ct in range(4):
                xc = xp.tile([128, N], BF16, tag="xc")
                xs[(s, ct)] = xc
        for ct in range(4):
            nc.sync.dma_start(out=xs[(0, ct)][:, 0:2048], in_=xd[0, ct, :, 0:2048])
        for ct in range(4):
            nc.sync.dma_start(out=xs[(0, ct)][:, 2048:N], in_=xd[0, ct, :, 2048:N])
        for s in range(1, BPC):
            for ct in range(4):
                nc.sync.dma_start(out=xs[(s, ct)], in_=xd[s, ct, :, :])

        mmbs, statss = {}, {}

        def emit_q(s, g):
            es = []
            for ot in range(4):
                psq = psqp.tile([128, 512], F32, tag="psq")
                for ct in range(4):
                    nc.tensor.matmul(
                        psq,
                        lhsT=qwt[:, ct, ts(ot, 128)],
                        rhs=xs[(s, ct)][:, ts(g, 512)],
                        start=(ct == 0),
                        stop=(ct == 3),
                    )
                e = ep.tile([128, 512], BF16, tag="e")
                nc.scalar.activation(out=e, in_=psq, func=AF.Exp)
                es.append(e)
            return es

        def emit_redfin(s, g, es):
            ps16 = ps16p.tile([40, 512], F32, tag="ps16")
            for ot in range(4):
                nc.tensor.matmul(
                    ps16,
                    lhsT=wm[:, s, ot, :],
                    rhs=es[ot],
                    start=(ot == 0),
                    stop=(ot == 3),
                )
            den_sb = rdp.tile([8, 512], F32, tag="densb")
            nc.scalar.copy(out=den_sb, in_=ps16[32:40, :])
            rden = rdp.tile([8, 512], F32, tag="rden")
            nc.vector.reciprocal_approx_fast(out=rden, in_=den_sb)
            stt = sttp.tile([8, 512], BF16, tag="stt")
            nc.vector.tensor_mul(stt, ps16[0:8, :], rden)
            mmb, stats = mmbs[s], statss[s]
            for ot in range(4):
                psf = psfp.tile([128, 512], F32, tag="psf")
                nc.tensor.matmul(
                    psf, lhsT=w2t[:, ot, :], rhs=stt, start=True, stop=True
                )
                dst = mmb[:, ot, ts(g, 512)]
                if (ot + g) % 2 == 0:
                    nc.scalar.copy(out=dst, in_=psf)
                else:
                    nc.vector.tensor_copy(dst, psf)
                nc.vector.bn_stats(out=stats[:, ot, g], in_=psf)

        def emit_epilogue(s):
            mmb, stats = mmbs[s], statss[s]
            mv8 = smallp.tile([128, 8], F32, tag="mv8")
            for ot in range(4):
                mv = smallp.tile([128, 2], F32, tag="mv")
                nc.vector.bn_aggr(out=mv, in_=stats[:, ot])
                m_ = mv8[:, ot : ot + 1]
                nc.vector.tensor_add(m_, mv[:, 0:1], outb_c[:, ot : ot + 1])
                msq = smallp.tile([128, 1], F32, tag="msq")
                nc.vector.tensor_mul(msq, m_, m_)
                nc.vector.tensor_add(mv8[:, 4 + ot : 5 + ot], mv[:, 1:2], msq)

            ps_tot = ps16p.tile([1, 8], F32, tag="ps16")
            nc.tensor.matmul(ps_tot, lhsT=ones_col, rhs=mv8, start=True, stop=True)
            rowt8 = rowp.tile([1, 8], F32, tag="rowt8")
            nc.scalar.copy(out=rowt8, in_=ps_tot)
            tt = rowp.tile([1, 2], F32, tag="tt")
            nc.vector.reduce_sum(
                out=tt, in_=rowt8.rearrange("p (a b) -> p a b", a=2), axis=AX
            )
            tt2 = rowp.tile([1, 2], F32, tag="tt2")
            nc.scalar.mul(out=tt2, in_=tt, mul=1.0 / C)  # {mu, E2}
            msq = rowp.tile([1, 1], F32, tag="msq")
            nc.vector.tensor_mul(msq, tt2[:, 0:1], tt2[:, 0:1])
            var = rowp.tile([1, 1], F32, tag="var")
            nc.vector.tensor_sub(var, tt2[:, 1:2], msq)
            sd = rowp.tile([1, 1], F32, tag="sd")
            nc.scalar.activation(out=sd, in_=var, func=AF.Sqrt, bias=EPS)
            rstd = rowp.tile([1, 1], F32, tag="rstd")
            nc.vector.reciprocal(rstd, sd)
            murow = rowp.tile([1, 2], F32, tag="murow")
            nc.vector.tensor_copy(murow[:, 0:1], tt2[:, 0:1])
            nc.vector.tensor_copy(murow[:, 1:2], rstd)
            ps_b = ps16p.tile([128, 2], F32, tag="ps16")
            nc.tensor.matmul(ps_b, lhsT=ones_row, rhs=murow, start=True, stop=True)
            msb = smallp.tile([128, 2], F32, tag="msb")
            nc.scalar.copy(out=msb, in_=ps_b)

            a_col = smallp.tile([128, 4], F32, tag="acol")
            nc.vector.tensor_scalar_mul(a_col, gng_c, msb[:, 1:2])
            t1 = smallp.tile([128, 4], F32, tag="t1")
            nc.vector.tensor_scalar(
                out=t1, in0=outb_c, scalar1=msb[:, 0:1], scalar2=None,
                op0=OP.subtract,
            )
            t2 = smallp.tile([128, 4], F32, tag="t2")
            nc.vector.tensor_mul(t2, a_col, t1)
            b2 = smallp.tile([128, 4], F32, tag="b2")
            nc.vector.tensor_add(b2, t2, gnb_c)

            for ot in range(4):
                stg = stgp.tile([128, N], BF16, tag="stg")
                nc.vector.tensor_scalar(
                    out=stg, in0=mmb[:, ot, :],
                    scalar1=a_col[:, ot : ot + 1], scalar2=b2[:, ot : ot + 1],
                    op0=OP.mult, op1=OP.add,
                )
                nc.sync.dma_start(out=outd[s, ot, :, :], in_=stg)

        # software pipeline: PE runs q(g+1) while ACT/DVE produce e/stt for g
        pend = None
        for s in range(BPC):
            for g in range(8):
                if g == 0:
                    mmb_t = mmp.tile([128, 4, N], BF16, tag="mmb")
                    stats_t = accp.tile([128, 4, 8, 6], F32, tag="stats")
                    mmbs[s] = mmb_t
                    statss[s] = stats_t
                es = emit_q(s, g)
                if pend is not None:
                    ps_, pg_, pes_ = pend
                    emit_redfin(ps_, pg_, pes_)
                    if pg_ == 7:
                        emit_epilogue(ps_)
                pend = (s, g, es)
        ps_, pg_, pes_ = pend
        emit_redfin(ps_, pg_, pes_)
        emit_epilogue(ps_)

    nc.finalize()
    return nc


_NC_CACHE = {}


def _get_nc(use_f32r=True):
    if "nc" not in _NC_CACHE:
        _NC_CACHE["nc"] = build_nc()
    return _NC_CACHE["nc"]


def make_in_maps(inputs):
    f32 = np.float32
    x = np.ascontiguousarray(inputs["x"], dtype=f32).reshape(B, C, N)
    y = np.ascontiguousarray(inputs["y"], dtype=f32).reshape(B, DIMY)
    k_w = np.asarray(inputs["k_w"], f32)
    v_w = np.asarray(inputs["v_w"], f32)
    to_q_w = np.asarray(inputs["to_q_w"], f32)
    to_k_w = np.asarray(inputs["to_k_w"], f32)
    to_v_w = np.asarray(inputs["to_v_w"], f32)
    out_w = np.asarray(inputs["out_w"], f32)
    out_b = np.asarray(inputs["out_b"], f32)
    gn_g = np.asarray(inputs["gn_g"], f32)
    gn_b = np.asarray(inputs["gn_b"], f32)

    # host precompute: per-sample softmax-weighted value vector w[b,hd], and
    # the collapsed output weight W2[o,h] (all O(weights)/O(y) work)
    ky = y @ k_w.T                                   # [B, C]
    vy = y @ v_w.T
    rs_k = to_k_w.sum(1)                             # [C]
    rs_v = to_v_w.sum(1)
    ez = np.exp(rs_k[None, :, None] * ky[:, None, :])          # [B, hd, j]
    wvec = (ez * vy[:, None, :]).sum(-1) / ez.sum(-1)          # [B, C]
    W2 = SCALE * (
        out_w.reshape(C, HEADS, DHEAD) * rs_v.reshape(HEADS, DHEAD)[None]
    ).sum(-1)                                        # [C, 8]

    # reduction masks: [B, 4ot, 128p, 40]; col j: w if head==j, col 32+j: 1
    # (cols 8-31 zero-padded so num lands at psum partitions 0-7 and den at
    # 32-39 -- engine partition reads must be 32-aligned)
    hd = np.arange(C)
    head = hd // DHEAD
    ot_i, p_i = hd // 128, hd % 128
    wmask = np.zeros((B, 4, 128, 40), f32)
    wmask[:, ot_i, p_i, head] = wvec
    wmask[:, ot_i, p_i, 32 + head] = 1.0

    qwt = np.ascontiguousarray(
        to_q_w.T.reshape(4, 128, C).transpose(1, 0, 2)
    ).astype(BF16NP)                                 # [128p, 4ct, 512o]
    w2t = np.ascontiguousarray(W2.T.reshape(HEADS, 4, 128)).astype(BF16NP)
    gcols = np.ascontiguousarray(
        np.stack(
            [out_b.reshape(4, 128).T, gn_g.reshape(4, 128).T,
             gn_b.reshape(4, 128).T],
            axis=2,
        )
    ).astype(f32)                                    # [128, 4, 3]

    xb = x.reshape(B, 4, 128, N).astype(BF16NP)
    in_maps = []
    for core in range(NCORES):
        s0 = core * BPC
        m = {
            "xb": np.ascontiguousarray(xb[s0 : s0 + BPC]),
            "wm": np.ascontiguousarray(
                wmask[s0 : s0 + BPC].transpose(2, 0, 1, 3)
            ).astype(BF16NP),                        # [128, BPC, 4, 16]
            "qwt": qwt,
            "w2t": w2t,
            "gcols": gcols,
        }
        in_maps.append(m)
    return in_maps


def kernel(**inputs):
    nc = _get_nc()
    res = run_bass_kernel_spmd(nc, make_in_maps(inputs), list(range(NCORES)))
    out = np.concatenate([r["out"] for r in res.results], axis=0)  # [B,4,128,N]
    return out.reshape(B, C, N).astype(np.float32).reshape(B, C, 64, 64)


if __name__ == "__main__":
    rng = np.random.default_rng(0)
    inputs = {
        "x": rng.standard_normal((B, C, 64, 64), dtype=np.float32),
        "y": rng.standard_normal((B, 1, 1, DIMY), dtype=np.float32),
        "k_w": rng.standard_normal((C, DIMY), dtype=np.float32) * 0.02,
        "v_w": rng.standard_normal((C, DIMY), dtype=np.float32) * 0.02,
        "to_q_w": rng.standard_normal((C, C), dtype=np.float32) * 0.02,
        "to_k_w": rng.standard_normal((C, C), dtype=np.float32) * 0.02,
        "to_v_w": rng.standard_normal((C, C), dtype=np.float32) * 0.02,
        "out_w": rng.standard_normal((C, C), dtype=np.float32) * 0.02,
        "out_b": np.zeros(C, np.float32),
        "gn_g": np.ones(C, np.float32),
        "gn_b": np.zeros(C, np.float32),
    }
    out = kernel(**inputs)
    print("kernel ran, out shape", out.shape, "std", out.std())
